# revision 17
# baseline (speedup 1.0000x reference)
"""Trainium2 Bass kernel for the NTN problem.

out[b,k,q,a] = sigmoid( q[b,q,:] @ w[k] @ da[b,a,:]
                        + Vq[k]@q[b,q,:] + Vd[k]@da[b,a,:] + b[k] )

B=64, K=16, Q=A=D=256.  Sharding: data-parallel over batch B across the
8 NeuronCores (8 batches per core); w/V/b replicated.

Per core, per (k, batch-pair):
  MM1 (TensorE, fp16): tmp[e, q|q'] = sum_d w[k,d,e]^T qT[d, q|q']   (N=512)
  DVE: tmp PSUM->SBUF (fp16) with per-partition bias +Vd[k,e] (folds Vd@da)
  MM2 (TensorE, fp16): out[q, a] = sum_e tmp[e,q]^T daT[e, a]
  ScalarE: sigmoid(psum + bias mq[b,k,q]) where mq = Vq@q + b (host-prepped),
  written as fp16 into per-(b, qtile, k-chunk) collect tiles; the host
  upcasts the returned fp16 output to fp32 (quantization err stays well
  inside the 2e-2 budget) which halves store traffic 32->16 MB/core.

Engine budget per core (measured): PE stream ~111.5us, Scalar (256
sigmoids x ~450ns) ~115us, DVE ~97us.  Scalar is saturated end-to-end,
so total time ~= first-ACT time + 115us + store tail; everything in the
head exists to pull first-ACT earlier and to keep the PE HAM clock-gate
(un-throttles only after one fully-busy free-running 3413ns window;
resets on PE idle) from lingering at half clock.

Head schedule (from NTFF traces): framework preamble to ~6.5us, first
DMA configs at ~6.8 (one per dma_start, ~0.7us serialized per issuing
engine), first packets ~8.3, ~350GB/s aggregate across 16 shared
engines.  All first-use tensors are host-packed so DMA runs are 1-2KB
(512B packets + straggler engines previously pushed wk0's semaphore to
11.9us).  Loads are h-split so MM1(k0,h0) needs only 256KB: scalar
queue carries q2_0h0, q2_0h1, vdt, mq; sync queue wk0, w1, da2_0h0,
da2_0h1, w2..w7 in consumer-deadline order (w1 ahead of da2 so a late
w1 can't idle the PE after MM1(k0) -- a late da2 just reorders MM2
behind MM1(k1), which the tile scheduler handles).  PE warm-up matmuls
on a RAW (dependency-free) uninitialized SBUF scratch run from the
start barrier (~6.8) and are sized to end right at first-load-ready
(~9.8): 12 x N=256 plus 4 x N=128 for fine granularity at the handoff.
k0's MM1 and DVE are h-split so the first sigmoid fires ~11.8.

Output chunks ship as one dma_start per k-chunk ([128, h, qt, cs, A]
collect tiles, [b,q,k,a] DRAM layout so (k,a) merge into 2KB runs), one
chunk per 4-8 k's on SP, with a tapered 2-queue flush at the very end
(h0 on SP, h1 on Scalar for the last two cs=1 chunks).
"""

import os
import sys
import types
from contextlib import ExitStack

if "/opt/trn_rl_repo" not in sys.path:
    sys.path.insert(0, "/opt/trn_rl_repo")

import numpy as np

import concourse.bass as bass
import concourse.tile as tile
from concourse import bacc, bass_utils, mybir

F32 = mybir.dt.float32
F16 = mybir.dt.float16
SIG = mybir.ActivationFunctionType.Sigmoid

NCORES = 8
B, Q, A, D, K = 64, 256, 256, 256, 16
E = D
BL = B // NCORES
NBP = BL // 2

N_WARM_BIG = 16
N_WARM_SMALL = 2


def _install_profshim():
    """Provide antenv.axon_hooks so trace=True works under axon (best-effort)."""
    try:
        if "antenv.axon_hooks" in sys.modules:
            return True
        import antenv

        mod = types.ModuleType("antenv.axon_hooks")
        holder = {}
        mod.set_axon_ntff_profile_hook = lambda h: holder.__setitem__("h", h)
        mod.get_axon_ntff_profile_hook = lambda: holder.get("h")
        sys.modules["antenv.axon_hooks"] = mod
        antenv.axon_hooks = mod
        from trn_agent_boot.trn_boot import _ntff_profile_via_ctypes

        hook = _ntff_profile_via_ctypes("/opt/axon/libaxon_pjrt.so")
        if hook is None:
            return False
        mod.set_axon_ntff_profile_hook(hook)
        return True
    except Exception:
        return False


def _build_ntn(tc: tile.TileContext, ctx: ExitStack, aps: dict):
    nc = tc.nc
    DC, ET, QT = D // 128, E // 128, Q // 128
    qt, dat, w, vdt, mq, out = (aps[n] for n in ("qt", "dat", "w", "vdt", "mq", "out"))

    w_pool = ctx.enter_context(tc.tile_pool(name="w", bufs=1))
    const_pool = ctx.enter_context(tc.tile_pool(name="const", bufs=1))
    q_pool = ctx.enter_context(tc.tile_pool(name="q", bufs=3))
    da_pool = ctx.enter_context(tc.tile_pool(name="da", bufs=3))
    tmp_pool = ctx.enter_context(tc.tile_pool(name="tmp", bufs=4))
    out_pool = ctx.enter_context(tc.tile_pool(name="out", bufs=6))
    # PSUM is exactly 8 banks, allocated bank-granular: ptmp 4 x [128,512]f32
    # + pout 4 x [128,256]f32.  A 4-deep ptmp ring (2 per k) stops
    # MM1(k+1,et1) from waiting on DVE's read of k's ptmp every k, which a
    # 3-ring (shared with pwarm) did.  pwarm borrows a pout buf instead --
    # it frees right after warm-up, long before the 4-group MM2 runway
    # matters.
    ptmp_pool = ctx.enter_context(tc.tile_pool(name="ptmp", bufs=4, space="PSUM"))
    pout_pool = ctx.enter_context(tc.tile_pool(name="pout", bufs=4, space="PSUM"))

    act_tiles = {}

    # PE warm-up: dummy matmuls on a RAW (bass-level, dependency-free) SBUF
    # scratch keep the PE continuously busy from the end of its sequencer
    # preamble until the first real loads land, so the HAM un-throttle
    # window (one fully-busy free-running 3413ns window; <=400ns of idle
    # inside a window is tolerated, more resets it) completes as early as
    # possible.  Reading uninitialized SBUF is fine (results land in a
    # write-only PSUM tile); a pool tile would need a runtime memset first.
    # Warm-up is sized for WORST-CASE load-ready (~12us) at HALF-clock
    # pace (~292ns per N=256 warm): when HAM fires early the warms run 2x
    # fast and the leftover gap is harmless (already un-throttled), but
    # when HAM is late an undershoot gap >400ns resets the un-throttle
    # window (measured: 614ns gap -> full clock only at 15.9us, +3.5us).
    # The PE queue is in-order, so warms cannot fill gaps behind a
    # waiting real matmul -- the count must cover the worst case up
    # front.  pwarm joins pout's ring so it doesn't shrink ptmp's.
    scratch_h = ctx.enter_context(nc.sbuf_tensor("warm_scratch", [128, 256], F16))
    scratch = scratch_h.ap()
    pwarm = pout_pool.tile([128, A], F32, name="pwarm", tag="po")
    for _ in range(N_WARM_BIG):
        nc.tensor.matmul(pwarm[:], lhsT=scratch[:, 0:128], rhs=scratch[:],
                         start=True, stop=True)
    for _ in range(N_WARM_SMALL):
        nc.tensor.matmul(pwarm[:, 0:128], lhsT=scratch[:, 0:128],
                         rhs=scratch[:, 0:128], start=True, stop=True)

    def load_pair(bp):
        q2 = q_pool.tile([128, DC, 2 * Q], F16, name=f"q2_{bp}", tag="q2")
        nc.sync.dma_start(q2[:], qt[bp])
        da2 = da_pool.tile([128, ET, 2 * A], F16, name=f"da2_{bp}", tag="da2")
        nc.sync.dma_start(da2[:], dat[bp])
        act_tiles[bp] = (q2, da2)

    def load_wk(k):
        wk = w_pool.tile([128, DC, E], F16, name=f"wk{k}", tag=f"wk{k}")
        nc.sync.dma_start(wk[:], w[k])
        return wk

    # Critical head loads, interleaved so each queue is in consumer-deadline
    # order.  Sync: wk0, w1, vdt, da2_0h0, mq, w2..w7 (w1 ahead of da2 so a
    # late w1 can't idle the PE right after MM1(k0)).  Scalar: q2_0h0,
    # q2_0h1, da2_0h1, then a dependency-free dummy sigmoid on the raw warm
    # scratch, which makes the table-insert pass place the sigmoid
    # ACT_TABLE_LOAD right there (~9.5) instead of behind the mq-region
    # wait where it would gate the first real ACT.
    w_sb = {}
    q2_0 = q_pool.tile([128, DC, 2 * Q], F16, name="q2_0", tag="q2")
    da2_0 = da_pool.tile([128, ET, 2 * A], F16, name="da2_0", tag="da2")
    vdt_sb = const_pool.tile([128, ET, K], F32)
    mq_sb = const_pool.tile([128, QT, BL, K], F16)
    w_sb[0] = load_wk(0)
    nc.scalar.dma_start(q2_0[:, :, 0:Q], qt[0][:, :, 0:Q])
    w_sb[1] = load_wk(1)
    nc.scalar.dma_start(q2_0[:, :, Q:2 * Q], qt[0][:, :, Q:2 * Q])
    nc.sync.dma_start(vdt_sb[:], vdt)
    nc.scalar.dma_start(da2_0[:, :, A:2 * A], dat[0][:, :, A:2 * A])
    nc.sync.dma_start(da2_0[:, :, 0:A], dat[0][:, :, 0:A])
    nc.sync.dma_start(mq_sb[:], mq)
    nc.scalar.activation(scratch[:, 4:5], scratch[:, 0:1], SIG,
                         bias=scratch[:, 2:3])
    act_tiles[0] = (q2_0, da2_0)
    for k in range(2, 8):
        w_sb[k] = load_wk(k)
    # w8..w15 are needed >=20us in, so they ride one bulk dma_start --
    # 7 fewer queue entries trims sequencer configs and end-of-kernel drain.
    w_bulk = w_pool.tile([128, 8, DC, E], F16, name="w_bulk", tag="w_bulk")
    nc.sync.dma_start(w_bulk[:], w[8:K].rearrange("k p dc e -> p k dc e"))
    for k in range(8, K):
        w_sb[k] = w_bulk[:, k - 8]

    # One collect tile per k-chunk covers both batches and both q-tiles
    # ([128, h, qt, cs, A]), so a chunk ships as a SINGLE dma_start (fewer
    # ~0.7us DGE configs serializing on the sequencers, and a shorter
    # end-of-kernel queue drain).  Chunks complete at their boundary k and
    # the store issues right there on the SP queue; the last two tapered
    # chunks split h across SP/Scalar so the final flush runs two configs
    # in parallel.
    for bp in range(NBP):
        b0, b1 = 2 * bp, 2 * bp + 1
        if bp not in act_tiles:
            load_pair(bp)
        if bp + 1 < NBP and bp + 1 not in act_tiles:
            load_pair(bp + 1)
        q2, da2 = act_tiles.pop(bp)

        last_bp = bp == NBP - 1
        if not last_bp:
            chunk_sizes = [8, 8]
        else:
            chunk_sizes = [4, 4, 4, 2, 1, 1]
        k2chunk = {}
        koff = 0
        for ci, cs in enumerate(chunk_sizes):
            for off in range(cs):
                k2chunk[koff + off] = (ci, off, cs)
            koff += cs
        coll = {ci: out_pool.tile([128, 2, QT, cs, A], F16, name="coll", tag="coll")
                for ci, cs in enumerate(chunk_sizes)}

        for k in range(K):
            ptmps = [ptmp_pool.tile([128, 2 * Q], F32, name=f"pt{et}", tag="pt")
                     for et in range(ET)]
            first_k = bp == 0 and k == 0
            if first_k:
                # h-split MM1 only at k0, so the first matmul needs just
                # q2_0h0+wk0 (256KB of loads).  Everywhere else MM1 stays
                # N=512: all-N=256 h-splitting was measured at 131ns/matmul
                # (stationary reload not hidden behind a 107ns stream),
                # +22us of PE time.  A 2-free-dim rhs AP to fuse the new
                # layout streamed columns in the wrong order (rel err ~1),
                # hence the h-merged [dc, (h q)] tile layout.
                for h in (0, 1):
                    for et in range(ET):
                        for dc in range(DC):
                            nc.tensor.matmul(
                                ptmps[et][:, h * Q:(h + 1) * Q],
                                lhsT=w_sb[k][:, dc, et * 128:(et + 1) * 128],
                                rhs=q2[:, dc, h * Q:(h + 1) * Q],
                                start=(dc == 0),
                                stop=(dc == DC - 1),
                            )
            else:
                for et in range(ET):
                    for dc in range(DC):
                        nc.tensor.matmul(
                            ptmps[et][:],
                            lhsT=w_sb[k][:, dc, et * 128:(et + 1) * 128],
                            rhs=q2[:, dc, :],
                            start=(dc == 0),
                            stop=(dc == DC - 1),
                        )
            # Both adds stay on DVE: offloading to GpSimd/Pool fails to
            # lower in walrus (the Pool engine can't take this
            # tensor_scalar form).
            tmp = tmp_pool.tile([128, ET, 2 * Q], F16)
            if first_k or (last_bp and k == K - 1):
                # h-split adds: at k0 so MM2(h0) starts before MM1(k0,h1)'s
                # psum is even written; at the final k so the end-of-kernel
                # sigmoid/store chain pulls in.
                for h in (0, 1):
                    for et in range(ET):
                        nc.vector.tensor_scalar_add(
                            tmp[:, et, h * Q:(h + 1) * Q],
                            ptmps[et][:, h * Q:(h + 1) * Q],
                            vdt_sb[:, et, k:k + 1],
                        )
            else:
                for et in range(ET):
                    nc.vector.tensor_scalar_add(
                        tmp[:, et, :], ptmps[et][:], vdt_sb[:, et, k:k + 1]
                    )
            for h, b in ((0, b0), (1, b1)):
                for qt_i in range(QT):
                    # Per-qt pout tiles: a double-wide [128, 2A] po shared by
                    # both qt's made the last k's MM2 wait on the sibling
                    # qt's still-draining ACT (measured 0.5+0.35us PE stalls
                    # at the tail); per-qt tiles decouple them.
                    po = pout_pool.tile([128, A], F32, name="po", tag="po")
                    for et in range(ET):
                        nc.tensor.matmul(
                            po[:],
                            lhsT=tmp[:, et, h * Q + qt_i * 128: h * Q + (qt_i + 1) * 128],
                            rhs=da2[:, et, h * A:(h + 1) * A],
                            start=(et == 0),
                            stop=(et == ET - 1),
                        )
                    nc.scalar.activation(
                        coll[k2chunk[k][0]][:, h, qt_i, k2chunk[k][1], :], po[:], SIG,
                        bias=mq_sb[:, qt_i, b, k:k + 1],
                    )
            ci, off, cs = k2chunk[k]
            if off == cs - 1:
                k_lo = k - cs + 1
                # out DRAM layout is [b, q, k, a] so (k, a) merges into one
                # contiguous 512*cs-byte run per (b, q) -- 3 free dims (DMA
                # AP limit) and fat descriptors; host restores [b, k, q, a].
                dram = out[b0:b0 + 2, :, k_lo:k_lo + cs].rearrange(
                    "h (qt p) k a -> p h qt k a", p=128)
                if last_bp and ci == len(chunk_sizes) - 1:
                    # final chunk: all configs on SP (a Scalar-queue config
                    # would occupy the Activation engine between the last ACT
                    # groups, delaying them); h1 per-qt so the post-last-ACT
                    # drain is only 64KB.
                    nc.sync.dma_start(dram[:, 0], coll[ci][:, 0])
                    nc.sync.dma_start(dram[:, 1, 0], coll[ci][:, 1, 0])
                    nc.sync.dma_start(dram[:, 1, 1], coll[ci][:, 1, 1])
                elif last_bp and ci == len(chunk_sizes) - 2:
                    nc.sync.dma_start(dram[:, 0], coll[ci][:, 0])
                    nc.sync.dma_start(dram[:, 1], coll[ci][:, 1])
                else:
                    nc.sync.dma_start(dram, coll[ci][:])


_COMPILED = None


def _get_compiled():
    global _COMPILED
    if _COMPILED is not None:
        return _COMPILED
    nc = bacc.Bacc("TRN2", target_bir_lowering=False, debug=False, num_devices=NCORES)
    aps = {
        "qt": nc.dram_tensor("qt", [NBP, 128, D // 128, 2 * Q], F16,
                             kind="ExternalInput").ap(),
        "dat": nc.dram_tensor("dat", [NBP, 128, E // 128, 2 * A], F16,
                              kind="ExternalInput").ap(),
        "w": nc.dram_tensor("w", [K, 128, D // 128, E], F16,
                            kind="ExternalInput").ap(),
        "vdt": nc.dram_tensor("vdt", [128, E // 128, K], F32,
                              kind="ExternalInput").ap(),
        "mq": nc.dram_tensor("mq", [128, Q // 128, BL, K], F16,
                             kind="ExternalInput").ap(),
        "out": nc.dram_tensor("out", [BL, Q, K, A], F16, kind="ExternalOutput").ap(),
    }
    with tile.TileContext(nc) as tc:
        with ExitStack() as ctx:
            _build_ntn(tc, ctx, aps)
    nc.compile()
    _COMPILED = nc
    return nc


def kernel(batch_q_em, batch_da_em, w, V, b):
    q = np.ascontiguousarray(np.asarray(batch_q_em, dtype=np.float32))
    da = np.ascontiguousarray(np.asarray(batch_da_em, dtype=np.float32))
    w = np.ascontiguousarray(np.asarray(w, dtype=np.float32))
    V = np.ascontiguousarray(np.asarray(V, dtype=np.float32))
    b = np.asarray(b, dtype=np.float32).reshape(-1)

    # All tensors are host-packed for fat DMA runs: qt/dat are the exact
    # SBUF tile image [bp, p, dc, (h q)] (2KB/partition contiguous for
    # whole-pair loads; the pair-0 h-half slices degrade to 512B packets
    # but only cover 256KB on the critical path).
    qt = np.ascontiguousarray(
        q.transpose(0, 2, 1).reshape(B, D // 128, 128, Q).transpose(0, 2, 1, 3)
        .reshape(B // 2, 2, 128, D // 128, Q).transpose(0, 2, 3, 1, 4)
        .reshape(B // 2, 128, D // 128, 2 * Q)
    ).astype(np.float16)
    dat = np.ascontiguousarray(
        da.transpose(0, 2, 1).reshape(B, E // 128, 128, A).transpose(0, 2, 1, 3)
        .reshape(B // 2, 2, 128, E // 128, A).transpose(0, 2, 3, 1, 4)
        .reshape(B // 2, 128, E // 128, 2 * A)
    ).astype(np.float16)
    # w: [k, 128, dc, e] -- per (k, partition) one 2KB run.
    w16 = np.ascontiguousarray(
        w.reshape(K, D // 128, 128, E).transpose(0, 2, 1, 3)
    ).astype(np.float16)
    # vdt: [128, et, k] -- per partition one 128B run.
    vdt = np.ascontiguousarray(
        V[:, D:].T.reshape(E // 128, 128, K).transpose(1, 0, 2)
    )
    # mq[b,q,k] = q[b] @ Vq^T + bias
    mqT = q @ V[:, :D].T + b[None, None, :]              # [B, Q, K]

    nc = _get_compiled()
    in_maps = []
    for c in range(NCORES):
        s = slice(c * BL, (c + 1) * BL)
        sp = slice(c * NBP, (c + 1) * NBP)
        # mq: [128, qt, b, k] -- per partition one 512B run.
        mq_shard = np.ascontiguousarray(
            mqT[s].reshape(BL, Q // 128, 128, K).transpose(2, 1, 0, 3)
        ).astype(np.float16)
        in_maps.append({
            "qt": np.ascontiguousarray(qt[sp]),
            "dat": np.ascontiguousarray(dat[sp]),
            "w": w16,
            "vdt": vdt,
            "mq": mq_shard,
        })

    trace = bool(int(os.environ.get("NTN_TRACE", "0"))) and _install_profshim()
    res = bass_utils.run_bass_kernel_spmd(
        nc, in_maps, core_ids=list(range(NCORES)), trace=trace
    )
    if trace and res.exec_time_ns is not None:
        print(f"HW exec time: {res.exec_time_ns} ns")
    out = np.concatenate([r["out"] for r in res.results], axis=0)  # [B, Q, K, A] f16
    return np.ascontiguousarray(out.transpose(0, 2, 1, 3), dtype=np.float32)


# revision 18
# speedup vs baseline: 1.0040x; 1.0040x over previous
"""Trainium2 Bass kernel for the NTN problem.

out[b,k,q,a] = sigmoid( q[b,q,:] @ w[k] @ da[b,a,:]
                        + Vq[k]@q[b,q,:] + Vd[k]@da[b,a,:] + b[k] )

B=64, K=16, Q=A=D=256.  Sharding: data-parallel over batch B across the
8 NeuronCores (8 batches per core); w/V/b replicated.

Per core, per (k, batch-pair):
  MM1 (TensorE, fp16): tmp[e, q|q'] = sum_d w[k,d,e]^T qT[d, q|q']   (N=512)
  DVE: tmp PSUM->SBUF (fp16) with per-partition bias +Vd[k,e] (folds Vd@da)
  MM2 (TensorE, fp16): out[q, a] = sum_e tmp[e,q]^T daT[e, a]
  ScalarE: sigmoid(psum + bias mq[b,k,q]) where mq = Vq@q + b (host-prepped),
  written as fp16 into per-(b, qtile, k-chunk) collect tiles; the host
  upcasts the returned fp16 output to fp32 (quantization err stays well
  inside the 2e-2 budget) which halves store traffic 32->16 MB/core.

Engine budget per core (measured): PE stream ~111.5us, Scalar (256
sigmoids x ~450ns) ~115us, DVE ~97us.  Scalar is saturated end-to-end,
so total time ~= first-ACT time + 115us + store tail; everything in the
head exists to pull first-ACT earlier and to keep the PE HAM clock-gate
(un-throttles only after one fully-busy free-running 3413ns window;
resets on PE idle) from lingering at half clock.

Head schedule (from NTFF traces): framework preamble to ~6.5us, first
DMA configs at ~6.8 (one per dma_start, ~0.7us serialized per issuing
engine), first packets ~8.3, ~350GB/s aggregate across 16 shared
engines.  All first-use tensors are host-packed so DMA runs are 1-2KB
(512B packets + straggler engines previously pushed wk0's semaphore to
11.9us).  Loads are h-split so MM1(k0,h0) needs only 256KB: scalar
queue carries q2_0h0, q2_0h1, vdt, mq; sync queue wk0, w1, da2_0h0,
da2_0h1, w2..w7 in consumer-deadline order (w1 ahead of da2 so a late
w1 can't idle the PE after MM1(k0) -- a late da2 just reorders MM2
behind MM1(k1), which the tile scheduler handles).  PE warm-up matmuls
on a RAW (dependency-free) uninitialized SBUF scratch run from the
start barrier (~6.8) and are sized to end right at first-load-ready
(~9.8): 12 x N=256 plus 4 x N=128 for fine granularity at the handoff.
k0's MM1 and DVE are h-split so the first sigmoid fires ~11.8.

Output chunks ship as one dma_start per k-chunk ([128, h, qt, cs, A]
collect tiles, [b,q,k,a] DRAM layout so (k,a) merge into 2KB runs), one
chunk per 4-8 k's on SP, with a tapered 2-queue flush at the very end
(h0 on SP, h1 on Scalar for the last two cs=1 chunks).
"""

import os
import sys
import types
from contextlib import ExitStack

if "/opt/trn_rl_repo" not in sys.path:
    sys.path.insert(0, "/opt/trn_rl_repo")

import numpy as np

import concourse.bass as bass
import concourse.tile as tile
from concourse import bacc, bass_utils, mybir

F32 = mybir.dt.float32
F16 = mybir.dt.float16
SIG = mybir.ActivationFunctionType.Sigmoid

NCORES = 8
B, Q, A, D, K = 64, 256, 256, 256, 16
E = D
BL = B // NCORES
NBP = BL // 2

N_WARM_BIG = 16
N_WARM_SMALL = 2


def _install_profshim():
    """Provide antenv.axon_hooks so trace=True works under axon (best-effort)."""
    try:
        if "antenv.axon_hooks" in sys.modules:
            return True
        import antenv

        mod = types.ModuleType("antenv.axon_hooks")
        holder = {}
        mod.set_axon_ntff_profile_hook = lambda h: holder.__setitem__("h", h)
        mod.get_axon_ntff_profile_hook = lambda: holder.get("h")
        sys.modules["antenv.axon_hooks"] = mod
        antenv.axon_hooks = mod
        from trn_agent_boot.trn_boot import _ntff_profile_via_ctypes

        hook = _ntff_profile_via_ctypes("/opt/axon/libaxon_pjrt.so")
        if hook is None:
            return False
        mod.set_axon_ntff_profile_hook(hook)
        return True
    except Exception:
        return False


def _build_ntn(tc: tile.TileContext, ctx: ExitStack, aps: dict):
    nc = tc.nc
    DC, ET, QT = D // 128, E // 128, Q // 128
    qt, dat, w, vdt, mq, out = (aps[n] for n in ("qt", "dat", "w", "vdt", "mq", "out"))

    w_pool = ctx.enter_context(tc.tile_pool(name="w", bufs=1))
    const_pool = ctx.enter_context(tc.tile_pool(name="const", bufs=1))
    q_pool = ctx.enter_context(tc.tile_pool(name="q", bufs=3))
    da_pool = ctx.enter_context(tc.tile_pool(name="da", bufs=3))
    tmp_pool = ctx.enter_context(tc.tile_pool(name="tmp", bufs=4))
    out_pool = ctx.enter_context(tc.tile_pool(name="out", bufs=6))
    # PSUM is exactly 8 banks, allocated bank-granular: ptmp 4 x [128,512]f32
    # + pout 4 x [128,256]f32.  A 4-deep ptmp ring (2 per k) stops
    # MM1(k+1,et1) from waiting on DVE's read of k's ptmp every k, which a
    # 3-ring (shared with pwarm) did.  pwarm borrows a pout buf instead --
    # it frees right after warm-up, long before the 4-group MM2 runway
    # matters.
    ptmp_pool = ctx.enter_context(tc.tile_pool(name="ptmp", bufs=4, space="PSUM"))
    pout_pool = ctx.enter_context(tc.tile_pool(name="pout", bufs=4, space="PSUM"))

    act_tiles = {}

    # PE warm-up: dummy matmuls on a RAW (bass-level, dependency-free) SBUF
    # scratch keep the PE continuously busy from the end of its sequencer
    # preamble until the first real loads land, so the HAM un-throttle
    # window (one fully-busy free-running 3413ns window; <=400ns of idle
    # inside a window is tolerated, more resets it) completes as early as
    # possible.  Reading uninitialized SBUF is fine (results land in a
    # write-only PSUM tile); a pool tile would need a runtime memset first.
    # Warm-up is sized for WORST-CASE load-ready (~12us) at HALF-clock
    # pace (~292ns per N=256 warm): when HAM fires early the warms run 2x
    # fast and the leftover gap is harmless (already un-throttled), but
    # when HAM is late an undershoot gap >400ns resets the un-throttle
    # window (measured: 614ns gap -> full clock only at 15.9us, +3.5us).
    # The PE queue is in-order, so warms cannot fill gaps behind a
    # waiting real matmul -- the count must cover the worst case up
    # front.  pwarm joins pout's ring so it doesn't shrink ptmp's.
    scratch_h = ctx.enter_context(nc.sbuf_tensor("warm_scratch", [128, 256], F16))
    scratch = scratch_h.ap()
    pwarm = pout_pool.tile([128, A], F32, name="pwarm", tag="po")
    for _ in range(N_WARM_BIG):
        nc.tensor.matmul(pwarm[:], lhsT=scratch[:, 0:128], rhs=scratch[:],
                         start=True, stop=True)
    for _ in range(N_WARM_SMALL):
        nc.tensor.matmul(pwarm[:, 0:128], lhsT=scratch[:, 0:128],
                         rhs=scratch[:, 0:128], start=True, stop=True)

    def load_pair(bp):
        q2 = q_pool.tile([128, DC, 2 * Q], F16, name=f"q2_{bp}", tag="q2")
        nc.sync.dma_start(q2[:], qt[bp])
        da2 = da_pool.tile([128, ET, 2 * A], F16, name=f"da2_{bp}", tag="da2")
        nc.sync.dma_start(da2[:], dat[bp])
        act_tiles[bp] = (q2, da2)

    def load_wk(k):
        wk = w_pool.tile([128, DC, E], F16, name=f"wk{k}", tag=f"wk{k}")
        nc.sync.dma_start(wk[:], w[k])
        return wk

    # Critical head loads, interleaved so each queue is in consumer-deadline
    # order.  Sync: wk0, w1, vdt, da2_0h0, mq, w2..w7 (w1 ahead of da2 so a
    # late w1 can't idle the PE right after MM1(k0)).  Scalar: q2_0h0,
    # q2_0h1, da2_0h1, then a dependency-free dummy sigmoid on the raw warm
    # scratch, which makes the table-insert pass place the sigmoid
    # ACT_TABLE_LOAD right there (~9.5) instead of behind the mq-region
    # wait where it would gate the first real ACT.
    w_sb = {}
    q2_0 = q_pool.tile([128, DC, 2 * Q], F16, name="q2_0", tag="q2")
    da2_0 = da_pool.tile([128, ET, 2 * A], F16, name="da2_0", tag="da2")
    vdt_sb = const_pool.tile([128, ET, K], F32)
    mq_sb = const_pool.tile([128, QT, BL, K], F16)
    w_sb[0] = load_wk(0)
    nc.scalar.dma_start(q2_0[:, :, 0:Q], qt[0][:, :, 0:Q])
    w_sb[1] = load_wk(1)
    nc.scalar.dma_start(q2_0[:, :, Q:2 * Q], qt[0][:, :, Q:2 * Q])
    nc.sync.dma_start(vdt_sb[:], vdt)
    nc.scalar.dma_start(da2_0[:, :, A:2 * A], dat[0][:, :, A:2 * A])
    nc.sync.dma_start(da2_0[:, :, 0:A], dat[0][:, :, 0:A])
    nc.sync.dma_start(mq_sb[:], mq)
    nc.scalar.activation(scratch[:, 4:5], scratch[:, 0:1], SIG,
                         bias=scratch[:, 2:3])
    act_tiles[0] = (q2_0, da2_0)
    for k in range(2, 8):
        w_sb[k] = load_wk(k)
    # w8..w15 are needed >=20us in, so they ride one bulk dma_start --
    # 7 fewer queue entries trims sequencer configs and end-of-kernel drain.
    w_bulk = w_pool.tile([128, 8, DC, E], F16, name="w_bulk", tag="w_bulk")
    nc.sync.dma_start(w_bulk[:], w[8:K].rearrange("k p dc e -> p k dc e"))
    for k in range(8, K):
        w_sb[k] = w_bulk[:, k - 8]

    # One collect tile per k-chunk covers both batches and both q-tiles
    # ([128, h, qt, cs, A]), so a chunk ships as a SINGLE dma_start (fewer
    # ~0.7us DGE configs serializing on the sequencers, and a shorter
    # end-of-kernel queue drain).  Chunks complete at their boundary k and
    # the store issues right there on the SP queue; the last two tapered
    # chunks split h across SP/Scalar so the final flush runs two configs
    # in parallel.
    for bp in range(NBP):
        b0, b1 = 2 * bp, 2 * bp + 1
        if bp not in act_tiles:
            load_pair(bp)
        if bp + 1 < NBP and bp + 1 not in act_tiles:
            load_pair(bp + 1)
        q2, da2 = act_tiles.pop(bp)

        last_bp = bp == NBP - 1
        if not last_bp:
            chunk_sizes = [8, 8]
        else:
            chunk_sizes = [4, 4, 4, 2, 1, 1]
        k2chunk = {}
        koff = 0
        for ci, cs in enumerate(chunk_sizes):
            for off in range(cs):
                k2chunk[koff + off] = (ci, off, cs)
            koff += cs
        coll = {ci: out_pool.tile([128, 2, QT, cs, A], F16, name="coll", tag="coll")
                for ci, cs in enumerate(chunk_sizes)}

        for k in range(K):
            ptmps = [ptmp_pool.tile([128, 2 * Q], F32, name=f"pt{et}", tag="pt")
                     for et in range(ET)]
            first_k = bp == 0 and k == 0
            if first_k:
                # h-split MM1 only at k0, so the first matmul needs just
                # q2_0h0+wk0 (256KB of loads).  Everywhere else MM1 stays
                # N=512: all-N=256 h-splitting was measured at 131ns/matmul
                # (stationary reload not hidden behind a 107ns stream),
                # +22us of PE time.  A 2-free-dim rhs AP to fuse the new
                # layout streamed columns in the wrong order (rel err ~1),
                # hence the h-merged [dc, (h q)] tile layout.
                for h in (0, 1):
                    for et in range(ET):
                        for dc in range(DC):
                            nc.tensor.matmul(
                                ptmps[et][:, h * Q:(h + 1) * Q],
                                lhsT=w_sb[k][:, dc, et * 128:(et + 1) * 128],
                                rhs=q2[:, dc, h * Q:(h + 1) * Q],
                                start=(dc == 0),
                                stop=(dc == DC - 1),
                            )
            else:
                for et in range(ET):
                    for dc in range(DC):
                        nc.tensor.matmul(
                            ptmps[et][:],
                            lhsT=w_sb[k][:, dc, et * 128:(et + 1) * 128],
                            rhs=q2[:, dc, :],
                            start=(dc == 0),
                            stop=(dc == DC - 1),
                        )
            # Both adds stay on DVE: offloading to GpSimd/Pool fails to
            # lower in walrus (the Pool engine can't take this
            # tensor_scalar form).
            tmp = tmp_pool.tile([128, ET, 2 * Q], F16)
            if first_k or (last_bp and k == K - 1):
                # h-split adds: at k0 so MM2(h0) starts before MM1(k0,h1)'s
                # psum is even written; at the final k so the end-of-kernel
                # sigmoid/store chain pulls in.
                for h in (0, 1):
                    for et in range(ET):
                        nc.vector.tensor_scalar_add(
                            tmp[:, et, h * Q:(h + 1) * Q],
                            ptmps[et][:, h * Q:(h + 1) * Q],
                            vdt_sb[:, et, k:k + 1],
                        )
            else:
                for et in range(ET):
                    nc.vector.tensor_scalar_add(
                        tmp[:, et, :], ptmps[et][:], vdt_sb[:, et, k:k + 1]
                    )
            for h, b in ((0, b0), (1, b1)):
                for qt_i in range(QT):
                    # Per-qt pout tiles: a double-wide [128, 2A] po shared by
                    # both qt's made the last k's MM2 wait on the sibling
                    # qt's still-draining ACT (measured 0.5+0.35us PE stalls
                    # at the tail); per-qt tiles decouple them.
                    po = pout_pool.tile([128, A], F32, name="po", tag="po")
                    for et in range(ET):
                        nc.tensor.matmul(
                            po[:],
                            lhsT=tmp[:, et, h * Q + qt_i * 128: h * Q + (qt_i + 1) * 128],
                            rhs=da2[:, et, h * A:(h + 1) * A],
                            start=(et == 0),
                            stop=(et == ET - 1),
                        )
                    nc.scalar.activation(
                        coll[k2chunk[k][0]][:, h, qt_i, k2chunk[k][1], :], po[:], SIG,
                        bias=mq_sb[:, qt_i, b, k:k + 1],
                    )
            ci, off, cs = k2chunk[k]
            if off == cs - 1:
                k_lo = k - cs + 1
                # out DRAM layout is [b, q, k, a] so (k, a) merges into one
                # contiguous 512*cs-byte run per (b, q) -- 3 free dims (DMA
                # AP limit) and fat descriptors; host restores [b, k, q, a].
                dram = out[b0:b0 + 2, :, k_lo:k_lo + cs].rearrange(
                    "h (qt p) k a -> p h qt k a", p=128)
                if last_bp and ci == len(chunk_sizes) - 1:
                    # Final chunk: all-SP routing serialized FOUR ~0.64us
                    # configs on the Sync engine after the last ACT (measured
                    # last packet at +3.2us).  Instead h1qt0 goes on Scalar,
                    # EMITTED AFTER all ACTs so it cannot delay any of them
                    # (the Activation engine is free then), and the two 64KB
                    # h1 entries drain in parallel on both queues.
                    nc.sync.dma_start(dram[:, 0], coll[ci][:, 0])
                    nc.scalar.dma_start(dram[:, 1, 0], coll[ci][:, 1, 0])
                    nc.sync.dma_start(dram[:, 1, 1], coll[ci][:, 1, 1])
                elif last_bp and ci == len(chunk_sizes) - 2:
                    nc.sync.dma_start(dram[:, 0], coll[ci][:, 0])
                    nc.sync.dma_start(dram[:, 1], coll[ci][:, 1])
                else:
                    nc.sync.dma_start(dram, coll[ci][:])


_COMPILED = None


def _get_compiled():
    global _COMPILED
    if _COMPILED is not None:
        return _COMPILED
    nc = bacc.Bacc("TRN2", target_bir_lowering=False, debug=False, num_devices=NCORES)
    aps = {
        "qt": nc.dram_tensor("qt", [NBP, 128, D // 128, 2 * Q], F16,
                             kind="ExternalInput").ap(),
        "dat": nc.dram_tensor("dat", [NBP, 128, E // 128, 2 * A], F16,
                              kind="ExternalInput").ap(),
        "w": nc.dram_tensor("w", [K, 128, D // 128, E], F16,
                            kind="ExternalInput").ap(),
        "vdt": nc.dram_tensor("vdt", [128, E // 128, K], F32,
                              kind="ExternalInput").ap(),
        "mq": nc.dram_tensor("mq", [128, Q // 128, BL, K], F16,
                             kind="ExternalInput").ap(),
        "out": nc.dram_tensor("out", [BL, Q, K, A], F16, kind="ExternalOutput").ap(),
    }
    with tile.TileContext(nc) as tc:
        with ExitStack() as ctx:
            _build_ntn(tc, ctx, aps)
    nc.compile()
    _COMPILED = nc
    return nc


def kernel(batch_q_em, batch_da_em, w, V, b):
    q = np.ascontiguousarray(np.asarray(batch_q_em, dtype=np.float32))
    da = np.ascontiguousarray(np.asarray(batch_da_em, dtype=np.float32))
    w = np.ascontiguousarray(np.asarray(w, dtype=np.float32))
    V = np.ascontiguousarray(np.asarray(V, dtype=np.float32))
    b = np.asarray(b, dtype=np.float32).reshape(-1)

    # All tensors are host-packed for fat DMA runs: qt/dat are the exact
    # SBUF tile image [bp, p, dc, (h q)] (2KB/partition contiguous for
    # whole-pair loads; the pair-0 h-half slices degrade to 512B packets
    # but only cover 256KB on the critical path).
    qt = np.ascontiguousarray(
        q.transpose(0, 2, 1).reshape(B, D // 128, 128, Q).transpose(0, 2, 1, 3)
        .reshape(B // 2, 2, 128, D // 128, Q).transpose(0, 2, 3, 1, 4)
        .reshape(B // 2, 128, D // 128, 2 * Q)
    ).astype(np.float16)
    dat = np.ascontiguousarray(
        da.transpose(0, 2, 1).reshape(B, E // 128, 128, A).transpose(0, 2, 1, 3)
        .reshape(B // 2, 2, 128, E // 128, A).transpose(0, 2, 3, 1, 4)
        .reshape(B // 2, 128, E // 128, 2 * A)
    ).astype(np.float16)
    # w: [k, 128, dc, e] -- per (k, partition) one 2KB run.
    w16 = np.ascontiguousarray(
        w.reshape(K, D // 128, 128, E).transpose(0, 2, 1, 3)
    ).astype(np.float16)
    # vdt: [128, et, k] -- per partition one 128B run.
    vdt = np.ascontiguousarray(
        V[:, D:].T.reshape(E // 128, 128, K).transpose(1, 0, 2)
    )
    # mq[b,q,k] = q[b] @ Vq^T + bias
    mqT = q @ V[:, :D].T + b[None, None, :]              # [B, Q, K]

    nc = _get_compiled()
    in_maps = []
    for c in range(NCORES):
        s = slice(c * BL, (c + 1) * BL)
        sp = slice(c * NBP, (c + 1) * NBP)
        # mq: [128, qt, b, k] -- per partition one 512B run.
        mq_shard = np.ascontiguousarray(
            mqT[s].reshape(BL, Q // 128, 128, K).transpose(2, 1, 0, 3)
        ).astype(np.float16)
        in_maps.append({
            "qt": np.ascontiguousarray(qt[sp]),
            "dat": np.ascontiguousarray(dat[sp]),
            "w": w16,
            "vdt": vdt,
            "mq": mq_shard,
        })

    trace = bool(int(os.environ.get("NTN_TRACE", "0"))) and _install_profshim()
    res = bass_utils.run_bass_kernel_spmd(
        nc, in_maps, core_ids=list(range(NCORES)), trace=trace
    )
    if trace and res.exec_time_ns is not None:
        print(f"HW exec time: {res.exec_time_ns} ns")
    out = np.concatenate([r["out"] for r in res.results], axis=0)  # [B, Q, K, A] f16
    return np.ascontiguousarray(out.transpose(0, 2, 1, 3), dtype=np.float32)
